# revision 1
# baseline (speedup 1.0000x reference)
"""Causal self-attention (RoPE) Trainium2 kernel, 8-way sharded.

Sharding: core = (batch b in 0..1) x (head group g in 0..3, 4 heads each).
Each core computes its batch's attention for its 4 heads plus the partial
output projection; the host sums the 4 partials per batch.

Layout strategy (per core):
- host passes xT = x[b].T (fp16) so the embed dim lands on SBUF partitions.
- W_qkv columns are permuted so q^T/k^T emerge from the projection matmul
  already transposed, with RoPE even/odd dim pairs de-interleaved into
  x1/x2 partition blocks (scores are invariant to a head-dim permutation).
- all matmul operands are fp16 (1 cycle/row on PE vs 4 for fp32); PSUM
  accumulation stays fp32. End-to-end error ~6e-4.
- scores are computed transposed (sT[j,i]); softmax needs no max pass
  (|scores| < ~4) and the denominator is obtained by appending a ones
  column to V (M=65 PV matmuls). Normalization per i-block via a selector
  matmul broadcast of 1/Z (DVE reciprocal_approx_fast).
- causal masking: only j<=i column ranges are computed; the diagonal
  128x128 block per j-tile gets a triangle multiply (Pool engine).

Scheduling strategy (the perf-critical part; Tile is a priority-based
list scheduler, so emission ORDER = priority):
- scores+exp are emitted one j-tile ahead of PV so exp(jt) on ACT hides
  under PV(jt-1)+scores(jt+1) on the PE.
- proj for chunk c+1 is emitted AFTER attention block c, so the scheduler
  treats attention as higher priority and BACKFILLS proj matmuls into the
  exp-gated PE gaps (this keeps the PE continuously busy, which also
  keeps it in its 2.4GHz p-state - idle drops it to 1.2GHz for 3us).
- the output projection is emitted last for the same reason: its matmuls
  backfill the exp-gaps of attention blocks 2-3.
- ACT does only exp (the attention-phase ceiling, ~80us); everything
  else is spread over DVE and Pool (NB: Pool cannot read PSUM and is
  ~3x slower than DVE per column - only SBUF-side, off-critical work).
- PSUM is exactly 8 banks: pa+pb (proj, 1 each; also reused by outproj
  and vproj) + scores ring (2x2 banks) + 2 ctx accumulators.
"""
import sys

sys.path.insert(0, "/opt/trn_rl_repo")

import numpy as np

NUM_HEADS = 16
HEAD_DIM = 64
B, S, E = 2, 2048, 1024
HG = 4                      # heads per core
NG = NUM_HEADS // HG        # head groups
N_CORES = B * NG
F_QK = 2 * HG * HEAD_DIM    # 512 projected q+k rows per core
F_V = HG * HEAD_DIM         # 256 v cols per core
ESUB = E // 128             # 8 K-subtiles over embed dim
NCHUNK = 4                  # 512-col seq chunks (projection)
CHUNK = S // NCHUNK         # 512
NST = S // 128              # 16 seq tiles of 128
BLK = 512                   # attention i-block width
NBLK = S // BLK             # 4

_CACHE = {}


def _build_program():
    import concourse.bass as bass
    import concourse.mybir as mybir
    import concourse.tile as tile
    from concourse import bacc

    f32 = mybir.dt.float32
    f16 = mybir.dt.float16
    Alu = mybir.AluOpType
    Act = mybir.ActivationFunctionType

    nc = bacc.Bacc("TRN2", target_bir_lowering=False, debug=False,
                   num_devices=N_CORES)

    xT_d = nc.dram_tensor("xT", (E, S), f16, kind="ExternalInput").ap()
    wqk_d = nc.dram_tensor("wqk", (E, F_QK), f16, kind="ExternalInput").ap()
    wv_d = nc.dram_tensor("wv", (E, F_V), f16, kind="ExternalInput").ap()
    wout_d = nc.dram_tensor("wout", (F_V, E), f16, kind="ExternalInput").ap()
    cs_d = nc.dram_tensor("cs", (128, S), f16, kind="ExternalInput").ap()
    sn_d = nc.dram_tensor("sn", (128, S), f16, kind="ExternalInput").ap()
    tri_d = nc.dram_tensor("tri", (128, 128), f16, kind="ExternalInput").ap()
    sel_d = nc.dram_tensor("sel", (4, 256), f16, kind="ExternalInput").ap()
    out_d = nc.dram_tensor("out", (S, E), f16, kind="ExternalOutput").ap()

    scale = 1.0 / float(np.sqrt(HEAD_DIM))

    with tile.TileContext(nc) as tc:
        with tc.tile_pool(name="wk", bufs=1) as wp, \
             tc.tile_pool(name="rsc", bufs=3) as rsc, \
             tc.tile_pool(name="pt", bufs=8) as ptp, \
             tc.tile_pool(name="sm", bufs=2) as smp, \
             tc.tile_pool(name="ot", bufs=8) as otp, \
             tc.tile_pool(name="pp", bufs=1, space="PSUM") as pp:
            # ---- persistent SBUF tensors ----
            xT_sb = wp.tile([128, ESUB, S], f16)
            wv_sb = wp.tile([128, ESUB, F_V], f16)
            wqk_sb = wp.tile([128, ESUB, F_QK], f16)
            wout_sb = wp.tile([128, 2, E], f16)
            cs_sb = wp.tile([128, S], f16)
            sn_sb = wp.tile([128, S], f16)
            tri_sb = wp.tile([128, 128], f16)
            sel_sb = wp.tile([4, 256], f16)
            v_sb = wp.tile([128, NST, HG * 65], f16)
            ctxu_sb = wp.tile([128, 2, S], f16)
            zall32 = wp.tile([4, S], f32)
            zall16 = wp.tile([4, S], f16)
            qra = wp.tile([128, S], f16)
            qrb = wp.tile([128, S], f16)
            kra = wp.tile([128, S], f16)
            krb = wp.tile([128, S], f16)
            qp = wp.tile([128, 2, S], f16)
            kp = wp.tile([128, 2, S], f16)

            # ---- input DMAs, ordered so vproj can start ASAP ----
            xT_r = xT_d.rearrange("(o p) s -> p o s", p=128)
            nc.sync.dma_start(wv_sb[:], wv_d.rearrange("(o p) f -> p o f", p=128))
            nc.sync.dma_start(xT_sb[:, :, 0:128], xT_r[:, :, 0:128])
            nc.sync.dma_start(xT_sb[:, :, 128:256], xT_r[:, :, 128:256])
            nc.sync.dma_start(xT_sb[:, :, 256:512], xT_r[:, :, 256:512])
            nc.sync.dma_start(wqk_sb[:], wqk_d.rearrange("(o p) f -> p o f", p=128))
            nc.sync.dma_start(cs_sb[:], cs_d[:])
            nc.sync.dma_start(sn_sb[:], sn_d[:])
            nc.sync.dma_start(xT_sb[:, :, CHUNK:2 * CHUNK],
                              xT_r[:, :, CHUNK:2 * CHUNK])
            nc.sync.dma_start(tri_sb[:], tri_d[:])
            nc.sync.dma_start(sel_sb[:], sel_d[:])

            # ones columns of v (only the 65th col of each head slot)
            nc.gpsimd.memset(
                v_sb[:].rearrange("p st (h w) -> p st h w", h=HG)[:, :, :, 64:65],
                1.0)

            # ---- emission helpers ----
            def emit_vproj(c):
                for st in range(4 * c, 4 * c + 4):
                    ssl = slice(st * 128, (st + 1) * 128)
                    pv = pp.tile([128, CHUNK], f32, tag="pa", name="pv")
                    for e in range(ESUB):
                        nc.tensor.matmul(pv[:, 0:F_V], xT_sb[:, e, ssl],
                                         wv_sb[:, e, :],
                                         start=(e == 0), stop=(e == ESUB - 1))
                    nc.vector.tensor_copy(
                        v_sb[:, st, :].rearrange("p (h w) -> p h w", h=HG)[:, :, 0:64],
                        pv[:, 0:F_V].rearrange("p (h w) -> p h w", h=HG))

            def emit_qkproj(c):
                csl = slice(c * CHUNK, (c + 1) * CHUNK)
                for (f0, ra, rb, dst) in ((0, qra, qrb, qp),
                                          (256, kra, krb, kp)):
                    pa = pp.tile([128, CHUNK], f32, tag="pa", name="pa")
                    pb = pp.tile([128, CHUNK], f32, tag="pb", name="pb")
                    for e in range(ESUB):
                        kw = dict(start=(e == 0), stop=(e == ESUB - 1))
                        xs = xT_sb[:, e, csl]
                        nc.tensor.matmul(pa[:], wqk_sb[:, e, f0:f0 + 128], xs, **kw)
                        nc.tensor.matmul(pb[:], wqk_sb[:, e, f0 + 128:f0 + 256], xs, **kw)
                    # rope: PSUM-reading mults on DVE, SBUF-only add/sub on Pool
                    t1 = rsc.tile([128, CHUNK], f32, tag="t1", name="t1")
                    t2 = rsc.tile([128, CHUNK], f32, tag="t2", name="t2")
                    nc.vector.tensor_tensor(t1[:], pa[:], cs_sb[:, csl], Alu.mult)
                    nc.vector.tensor_tensor(t2[:], pb[:], sn_sb[:, csl], Alu.mult)
                    nc.gpsimd.tensor_tensor(ra[:, csl], t1[:], t2[:], Alu.subtract)
                    t3 = rsc.tile([128, CHUNK], f32, tag="t1", name="t3")
                    t4 = rsc.tile([128, CHUNK], f32, tag="t2", name="t4")
                    nc.vector.tensor_tensor(t3[:], pa[:], sn_sb[:, csl], Alu.mult)
                    nc.vector.tensor_tensor(t4[:], pb[:], cs_sb[:, csl], Alu.mult)
                    nc.gpsimd.tensor_tensor(rb[:, csl], t3[:], t4[:], Alu.add)
                    for p in range(2):
                        h0, h1 = 2 * p, 2 * p + 1
                        nc.sync.dma_start(dst[0:32, p, csl],
                                          ra[32 * h0:32 * h0 + 32, csl])
                        nc.sync.dma_start(dst[32:64, p, csl],
                                          rb[32 * h0:32 * h0 + 32, csl])
                        nc.sync.dma_start(dst[64:96, p, csl],
                                          ra[32 * h1:32 * h1 + 32, csl])
                        nc.sync.dma_start(dst[96:128, p, csl],
                                          rb[32 * h1:32 * h1 + 32, csl])

            def emit_attn(bb):
                # scores+exp run one j-tile ahead of PV so exp(jt) hides
                # under PV(jt-1)+scores(jt+1) on the PE
                i0 = bb * BLK
                njt = 4 * bb + 4

                def s_and_e(p, jt):
                    r = jt - 4 * bb
                    off = 128 * max(r, 0)
                    ps_s = pp.tile([128, 2, BLK], f32, tag="s",
                                   name="ps_s", bufs=2)
                    for a in range(2):
                        nc.tensor.matmul(
                            ps_s[:, a, off:],
                            kp[64 * a:64 * a + 64, p,
                               128 * jt:128 * jt + 128],
                            qp[64 * a:64 * a + 64, p,
                               i0 + off:i0 + BLK],
                            start=True, stop=True)
                    pt = ptp.tile([128, 2, BLK], f16, tag="pt", name="pt")
                    nc.scalar.activation(pt[:, :, off:], ps_s[:, :, off:],
                                         Act.Exp, scale=scale)
                    if r >= 0:
                        # triangle mask on DVE: it sits on the exp->PV
                        # critical path, where Pool's latency is exposed
                        nc.vector.tensor_tensor(
                            pt[:, :, off:off + 128],
                            pt[:, :, off:off + 128],
                            tri_sb[:, None, :].to_broadcast((128, 2, 128)),
                            Alu.mult)
                    return pt, off

                for p in range(2):
                    ctx = [pp.tile([65, BLK], f32, tag=f"ctx{a}",
                                   name=f"ctx{a}") for a in range(2)]

                    def pv(jt, pt, off):
                        # per-element has_written handles the ragged causal
                        # ranges; the 2KB-granular group check cannot
                        for a in range(2):
                            nc.tensor.matmul(
                                ctx[a][:, off:],
                                v_sb[:, jt, 65 * (2 * p + a):
                                     65 * (2 * p + a) + 65],
                                pt[:, a, off:],
                                start=(jt == 0), stop=(jt == njt - 1),
                                skip_group_check=True)

                    prev = None
                    for jt in range(njt):
                        cur = s_and_e(p, jt)
                        if prev is not None:
                            pv(jt - 1, *prev)
                        prev = cur
                    pv(njt - 1, *prev)
                    # stash unnormalized ctx + Z rows (DVE)
                    for a in range(2):
                        nc.vector.tensor_copy(
                            ctxu_sb[64 * a:64 * a + 64, p, i0:i0 + BLK],
                            ctx[a][0:64, :])
                        zst = smp.tile([1, BLK], f32, tag="zst", name="zst",
                                       bufs=4)
                        nc.vector.tensor_copy(zst[:], ctx[a][64:65, :])
                        nc.sync.dma_start(
                            zall32[2 * p + a:2 * p + a + 1, i0:i0 + BLK],
                            zst[:])
                    if bb == NBLK - 1 and p == 0:
                        # last block: normalize pair 0 early, hidden under
                        # pair 1's attention, so the end-of-kernel norm chain
                        # only covers pair 1
                        qsl = slice(i0, i0 + BLK)
                        rz2 = smp.tile([2, BLK], f32, tag="rz2", name="rz2")
                        nc.vector.reciprocal_approx_fast(rz2[:],
                                                         zall32[0:2, qsl])
                        rz2h = smp.tile([2, BLK], f16, tag="rz2h", name="rz2h")
                        nc.vector.tensor_copy(rz2h[:], rz2[:])
                        zt0 = pp.tile([128, CHUNK], f32, tag="pa", name="zt0")
                        nc.tensor.matmul(zt0[:], sel_sb[0:2, 0:128], rz2h[:],
                                         start=True, stop=True)
                        nc.vector.tensor_tensor(ctxu_sb[:, 0, qsl],
                                                ctxu_sb[:, 0, qsl],
                                                zt0[:], Alu.mult)

            def emit_norm(bb):
                qsl = slice(bb * BLK, (bb + 1) * BLK)
                rz = smp.tile([4, BLK], f32, tag="rz", name="rz")
                nc.vector.reciprocal_approx_fast(rz[:], zall32[:, qsl])
                # Pool is ~3x slower per column: fine mid-stream (hidden by
                # the next block), but on the serial tail use DVE
                if bb == NBLK - 1:
                    nc.vector.tensor_copy(zall16[:, qsl], rz[:])
                else:
                    nc.gpsimd.tensor_copy(zall16[:, qsl], rz[:])
                for p in ([1] if bb == NBLK - 1 else range(2)):
                    zt = pp.tile([128, 2, BLK], f32, tag="s", name="zt",
                                 bufs=2)
                    nc.tensor.matmul(zt[:, 0, :],
                                     sel_sb[:, 128 * p:128 * p + 128],
                                     zall16[:, qsl], start=True, stop=True)
                    nc.vector.tensor_tensor(ctxu_sb[:, p, qsl],
                                            ctxu_sb[:, p, qsl],
                                            zt[:, 0, :], Alu.mult)

            def emit_outproj(bb, late):
                # po on pa/pb tags (ring 2, free once projections complete);
                # emitted LAST so these matmuls backfill attention exp-gaps
                for k in range(4):
                    st = 4 * bb + k
                    ssl = slice(st * 128, (st + 1) * 128)
                    for n in range(2):
                        i = 2 * k + n
                        if late and i % 4 >= 2:
                            po = pp.tile([128, 2, BLK], f32, tag="s",
                                         name="po2", bufs=2)[:, 0, :]
                        else:
                            po = pp.tile([128, CHUNK], f32,
                                         tag=("pa" if i % 2 == 0 else "pb"),
                                         name="po")[:]
                        nsl = slice(n * 512, (n + 1) * 512)
                        nc.tensor.matmul(po, ctxu_sb[:, 0, ssl],
                                         wout_sb[:, 0, nsl],
                                         start=True, stop=False)
                        nc.tensor.matmul(po, ctxu_sb[:, 1, ssl],
                                         wout_sb[:, 1, nsl],
                                         start=False, stop=True)
                        ot = otp.tile([128, 512], f16, tag="ot", name="ot")
                        if late:
                            nc.vector.tensor_copy(ot[:, 0:256], po[:, 0:256])
                            nc.scalar.copy(ot[:, 256:512], po[:, 256:512])
                        else:
                            nc.vector.tensor_copy(ot[:], po[:])
                        nc.sync.dma_start(out_d[ssl, nsl], ot[:])

            # ---- main stream ----
            emit_vproj(0)
            emit_qkproj(0)
            for bb in range(NBLK):
                emit_attn(bb)
                emit_norm(bb)
                if bb >= 1:
                    # outproj(bb-1) slots into the tag rings HERE, so its
                    # matmuls are allocation-ready at the block boundary and
                    # keep the PE busy (and p-state ramped) while the next
                    # chunk's rope/relayout latency resolves
                    emit_outproj(bb - 1, late=False)
                c = bb + 1
                if 1 <= c < NCHUNK:
                    emit_vproj(c)
                    emit_qkproj(c)
                    if c == 1:
                        nc.sync.dma_start(
                            xT_sb[:, :, 2 * CHUNK:3 * CHUNK],
                            xT_r[:, :, 2 * CHUNK:3 * CHUNK])
                        nc.sync.dma_start(
                            wout_sb[:],
                            wout_d.rearrange("(o p) e -> p o e", p=128))
                    if c == 2:
                        nc.sync.dma_start(
                            xT_sb[:, :, 3 * CHUNK:4 * CHUNK],
                            xT_r[:, :, 3 * CHUNK:4 * CHUNK])
            emit_outproj(NBLK - 1, late=True)

    nc.compile()
    return nc


def _host_inputs(x, W_qkv, W_out):
    """Build the 8 per-core input maps."""
    x = np.asarray(x, dtype=np.float32)
    W_qkv = np.asarray(W_qkv, dtype=np.float32)
    W_out = np.asarray(W_out, dtype=np.float32)

    pos = np.arange(S)
    freqs = 1.0 / 10000.0 ** (np.arange(0, HEAD_DIM, 2) / HEAD_DIM)
    ang = pos[:, None] * freqs[None, :]            # (S, 32)
    cs32 = np.cos(ang).T.astype(np.float32)        # (32, S)
    sn32 = np.sin(ang).T.astype(np.float32)
    cs = np.tile(cs32, (4, 1)).astype(np.float16)  # (128, S)
    sn = np.tile(sn32, (4, 1)).astype(np.float16)
    tri = (np.arange(128)[:, None] <= np.arange(128)[None, :]).astype(np.float16)
    # selector for Z broadcast: sel[k, 128p+m] = 1 where k == 2p + m//64
    sel = np.zeros((4, 256), np.float16)
    for p in range(2):
        for m in range(128):
            sel[2 * p + m // 64, 128 * p + m] = 1.0

    in_maps = []
    for b in range(B):
        xT = np.ascontiguousarray(x[b].T.astype(np.float16))
        for g in range(NG):
            heads = np.arange(HG * g, HG * g + HG)
            qa = np.concatenate([0 * NUM_HEADS * HEAD_DIM + h * HEAD_DIM
                                 + np.arange(0, HEAD_DIM, 2) for h in heads])
            qb = qa + 1
            ka = qa + NUM_HEADS * HEAD_DIM
            kb = ka + 1
            wqk = np.ascontiguousarray(
                W_qkv[:, np.concatenate([qa, qb, ka, kb])].astype(np.float16))
            vcols = np.concatenate([2 * NUM_HEADS * HEAD_DIM + h * HEAD_DIM
                                    + np.arange(HEAD_DIM) for h in heads])
            wv = np.ascontiguousarray(W_qkv[:, vcols].astype(np.float16))
            wout = np.ascontiguousarray(
                W_out[HG * g * HEAD_DIM:HG * (g + 1) * HEAD_DIM].astype(np.float16))
            in_maps.append({"xT": xT, "wqk": wqk, "wv": wv, "wout": wout,
                            "cs": cs, "sn": sn, "tri": tri, "sel": sel})
    return in_maps


def get_program():
    if "nc" not in _CACHE:
        _CACHE["nc"] = _build_program()
    return _CACHE["nc"]


def run(x, W_qkv, W_out, trace=False, tmpdir=None):
    from concourse import bass_utils
    nc = get_program()
    in_maps = _host_inputs(x, W_qkv, W_out)
    res = bass_utils.run_bass_kernel_spmd(
        nc, in_maps, core_ids=list(range(N_CORES)), trace=trace, tmpdir=tmpdir)
    out = np.zeros((B, S, E), np.float32)
    for b in range(B):
        for g in range(NG):
            out[b] += res.results[b * NG + g]["out"].astype(np.float32)
    return out, res


def kernel(x, W_qkv, W_out):
    out, _ = run(x, W_qkv, W_out)
    return out

